# revision 17
# baseline (speedup 1.0000x reference)
"""v7: scores = x^T (Wq Wk^T) x via on-device M/G precompute (no q/k
projections), 58-col q compaction, 2-pair merged tail ops, consolidated
DMA dispatches, fp16 output. Data-parallel over batch (1 core / element).

Zero-bias fast path: reference.setup_inputs() fixes bq=bk=bv=0. If any
bias is nonzero at runtime we fall back to an exact numpy implementation.
"""

import numpy as np

B, C, H, W, K = 8, 64, 64, 64, 7
HC = WC = H - K + 1          # 58
N = HC * WC                  # 3364
NPAIR = HC // 2              # 29 window-row pairs
NG = 15                      # 14 groups of 2 pairs + 1 single (pair 28)
LAG = 2                      # tail lag in groups
SCALE = float(1.0 / np.sqrt(C))
# slot index in the [128, 8, 116] scores tile for (pair parity, chunk):
# lo-half matmuls (chunks 0, 2) write bank A (slots 0-3), hi-half
# (chunks 1, 3) bank B (slots 4-7).
SLOT = [[0, 4, 1, 5], [2, 6, 3, 7]]

_CACHE = {}


def _pairs_of(g):
    return [2 * g, 2 * g + 1] if g < NG - 1 else [2 * g]


def _build_mask_np():
    """[128, 8, 116] band mask: kk = in-chunk k pixel (rp*64+jp),
    slot = SLOT[pp][c], q = jb*58 + j."""
    m = np.zeros((128, 8, 116), np.float16)
    kk = np.arange(128)
    rp, jp = kk // 64, kk % 64
    jq = np.arange(116)
    jb, j = jq // 58, jq % 58
    for pp in range(2):
        for c in range(4):
            di = 2 * c + rp[:, None] - jb[None, :]
            dj = jp[:, None] - j[None, :]
            ok = (di >= 0) & (di < K) & (dj >= 0) & (dj < K)
            m[:, SLOT[pp][c], :] = ok
    return np.ascontiguousarray(m)


def _build_module():
    import concourse.tile as tile
    from concourse import bacc, mybir

    dt = mybir.dt
    f32 = dt.float32
    f16 = dt.float16

    nc = bacc.Bacc(
        "TRN2", target_bir_lowering=False, debug=False, enable_asserts=False,
        num_devices=8,
    )

    x_d = nc.dram_tensor("x", [C, H, W], f16, kind="ExternalInput").ap()
    wqt_d = nc.dram_tensor("wqT", [C, C], f16, kind="ExternalInput").ap()
    wkt_d = nc.dram_tensor("wkT", [C, C], f16, kind="ExternalInput").ap()
    wv_d = nc.dram_tensor("wv", [C, C], f16, kind="ExternalInput").ap()
    mask_d = nc.dram_tensor("mask", [128, 8, 116], f16, kind="ExternalInput").ap()
    out_d = nc.dram_tensor("out", [N, C], f16, kind="ExternalOutput").ap()

    with tile.TileContext(nc) as tc:
        with (
            tc.tile_pool(name="const", bufs=1) as const,
            tc.tile_pool(name="attn", bufs=2) as attn,
            tc.tile_pool(name="fin", bufs=2) as fin,
            tc.tile_pool(name="ps", bufs=3, space="PSUM") as ps,
        ):
            x_sb = const.tile([128, H, W], f16)      # lo = x, hi = dup
            g_sb = const.tile([128, H, W], f16)      # G = M^T x, lo/hi dup
            v_sb = const.tile([128, 32, C + 1], f16)  # col 64 = ones (denom)
            mask_sb = const.tile([128, 8, 116], f16)
            wqt_sb = const.tile([C, C], f16)
            wkt_sb = const.tile([C, C], f16)
            wv_sb = const.tile([C, C], f16)
            m_sb = const.tile([C, C], f16)
            outt = const.tile([128, NPAIR, C], f16)

            # ---- input DMA dispatches ----
            # Critical-path inputs first (wv, wqT, wkT, x-lo); the bulk
            # transfers (x-hi dup, mask) are dispatched later so they don't
            # hog the DMA engines while the PE waits to start.
            nc.sync.dma_start(wv_sb[:], wv_d[:])
            nc.sync.dma_start(wqt_sb[:], wqt_d[:])
            nc.scalar.dma_start(wkt_sb[:], wkt_d[:])
            nc.sync.dma_start(x_sb[0:64, 0:32, :], x_d[:, 0:32, :])
            nc.sync.dma_start(x_sb[0:64, 32:64, :], x_d[:, 32:64, :])
            nc.gpsimd.memset(v_sb[:], 1.0)
            nc.gpsimd.dma_start(mask_sb[:], mask_d[:])

            # ---- M = Wq Wk^T ----
            pm = ps.tile([128, 8, 128], f32, tag="sc")
            pmf = pm[:].rearrange("p a b -> p (a b)")
            nc.tensor.matmul(pmf[0:64, 0:64], wqt_sb[:], wkt_sb[:])
            nc.scalar.copy(m_sb[:], pmf[0:64, 0:64])
            nc.scalar.dma_start(x_sb[64:128, 0:32, :], x_d[:, 0:32, :])
            nc.scalar.dma_start(x_sb[64:128, 32:64, :], x_d[:, 32:64, :])

            def emit_gchunk(gg):
                # G chunk gg = M^T x rows 8gg..8gg+7, then dup to hi partitions
                pg = ps.tile([128, 8, 128], f32, tag="sc")
                pgf = pg[:].rearrange("p a b -> p (a b)")
                nc.tensor.matmul(
                    pgf[0:64, 0:512], m_sb[:],
                    x_sb[0:64, 8 * gg:8 * gg + 8, :],
                )
                eng = nc.scalar.copy if gg % 2 == 0 else nc.vector.tensor_copy
                eng(g_sb[0:64, 8 * gg:8 * gg + 8, :], pgf[0:64, 0:512])
                nc.sync.dma_start(
                    g_sb[64:128, 8 * gg:8 * gg + 8, :],
                    g_sb[0:64, 8 * gg:8 * gg + 8, :],
                )

            for gg in range(8):
                emit_gchunk(gg)

            sctiles = [None] * NG

            def emit_vpair(r2):
                # project v for 2-row-pair chunks 2*r2, 2*r2+1
                pv = ps.tile([128, 2, C + 1], f32, tag="out", bufs=2)
                for h in range(2):
                    r = 2 * r2 + h
                    nc.tensor.matmul(
                        pv[:, h, 0:C],
                        x_sb[0:64, 2 * r:2 * r + 2, :],
                        wv_sb[:],
                    )
                eng = nc.scalar.copy if r2 % 2 == 0 else nc.vector.tensor_copy
                eng(v_sb[:, 2 * r2:2 * r2 + 2, 0:C], pv[:, :, 0:C])

            def emit_scores(g):
                sc = ps.tile([128, 8, 128], f32, tag="sc")
                sctiles[g] = sc
                for pp, p in enumerate(_pairs_of(g)):
                    i = 2 * p
                    glo = g_sb[0:64, i + 3:i + 5, 3:61]
                    ghi = g_sb[64:128, i + 3:i + 5, 3:61]
                    for c in range(4):
                        if c % 2 == 0:
                            nc.tensor.matmul(
                                sc[:, SLOT[pp][c], 0:116],
                                x_sb[0:64, i + 2 * c:i + 2 * c + 2, :],
                                glo,
                            )
                        else:
                            nc.tensor.matmul(
                                sc[:, SLOT[pp][c], 0:116],
                                x_sb[64:128, i + 2 * c:i + 2 * c + 2, :],
                                ghi,
                            )

            def emit_tail(g):
                sc = sctiles[g]
                pairs = _pairs_of(g)
                ex = attn.tile([128, 8, 116], f16, tag="ex")
                if len(pairs) == 2:
                    nc.scalar.activation(
                        ex[:], sc[:, :, 0:116],
                        mybir.ActivationFunctionType.Exp, scale=SCALE,
                    )
                else:
                    sc_h = sc[:].rearrange("p (h s) q -> p h s q", h=2)[:, :, 0:2, 0:116]
                    ex_h = ex[:].rearrange("p (h s) q -> p h s q", h=2)[:, :, 0:2, :]
                    nc.scalar.activation(
                        ex_h, sc_h,
                        mybir.ActivationFunctionType.Exp, scale=SCALE,
                    )
                at = attn.tile([128, 8, 116], f16, tag="at")
                if len(pairs) == 2:
                    nc.vector.tensor_mul(at[:], ex[:], mask_sb[:])
                else:
                    def _h(ap):
                        return ap.rearrange(
                            "p (h s) q -> p h s q", h=2)[:, :, 0:2, :]
                    nc.vector.tensor_mul(_h(at[:]), _h(ex[:]), _h(mask_sb[:]))
                ops = ps.tile([128, 2, C + 1], f32, tag="out", bufs=2)
                for pp, p in enumerate(pairs):
                    for c in range(4):
                        nc.tensor.matmul(
                            ops[0:116, pp, :],
                            at[:, SLOT[pp][c], :],
                            v_sb[:, p + c, :],
                            start=(c == 0), stop=(c == 3),
                        )
                rc = fin.tile([128, 2], f32, tag="rc")
                nc.vector.reciprocal(
                    rc[0:116, 0:len(pairs)],
                    ops[0:116, 0:len(pairs), C:C + 1].rearrange("p a b -> p (a b)"),
                )
                for pp, p in enumerate(pairs):
                    nc.vector.tensor_scalar(
                        outt[0:116, p:p + 1, :],
                        ops[0:116, pp:pp + 1, 0:C],
                        rc[0:116, pp:pp + 1], None,
                        mybir.AluOpType.mult,
                    )

            def emit_outdma(p0, p1):
                npair = p1 - p0
                dst = out_d[p0 * 2 * WC:p1 * 2 * WC, :].rearrange(
                    "(p jb j) c -> (jb j) p c", jb=2, j=WC,
                )
                nc.sync.dma_start(dst, outt[0:116, p0:p1, :])

            for g in range(NG):
                if g <= 15:
                    emit_vpair(g)
                if g == 14:
                    emit_vpair(15)
                emit_scores(g)
                if g >= LAG:
                    emit_tail(g - LAG)
                if g == 8:
                    emit_outdma(0, 14)
                if g == 13:
                    emit_outdma(14, 24)
            for g in range(NG - LAG, NG):
                emit_tail(g)
            emit_outdma(24, NPAIR)

    nc.compile()
    return nc


def _get_module():
    if "nc" not in _CACHE:
        _CACHE["nc"] = _build_module()
        _CACHE["mask"] = _build_mask_np()
    return _CACHE["nc"], _CACHE["mask"]


def _numpy_fallback(x, Wq, bq, Wk, bk, Wv, bv):
    xf = np.transpose(x, (0, 2, 3, 1)).reshape(B, H * W, C).astype(np.float64)
    ii, jj = np.meshgrid(np.arange(HC), np.arange(WC), indexing="ij")
    di, dj = np.meshgrid(np.arange(K), np.arange(K), indexing="ij")
    rows = ii.reshape(-1, 1) + di.reshape(1, -1)
    cols = jj.reshape(-1, 1) + dj.reshape(1, -1)
    idx = (rows * W + cols).reshape(-1)
    KK = K * K
    patches = xf[:, idx, :].reshape(B, N, KK, C)
    q = patches[:, :, KK // 2, :] @ Wq + bq
    k = patches @ Wk + bk
    v = patches @ Wv + bv
    s = np.einsum("bnc,bnkc->bnk", q, k) * SCALE
    a = np.exp(s - s.max(-1, keepdims=True))
    a /= a.sum(-1, keepdims=True)
    out = np.einsum("bnk,bnkc->bnc", a, v)
    return out.reshape(B, HC, WC, C).astype(np.float32)


def _make_in_maps(x, Wq, Wk, Wv, mask):
    wqt = np.ascontiguousarray(np.asarray(Wq).T).astype(np.float16)
    wkt = np.ascontiguousarray(np.asarray(Wk).T).astype(np.float16)
    wv = np.ascontiguousarray(np.asarray(Wv)).astype(np.float16)
    in_maps = []
    for b in range(B):
        in_maps.append({
            "x": np.ascontiguousarray(np.asarray(x[b]).astype(np.float16)),
            "wqT": wqt, "wkT": wkt, "wv": wv,
            "mask": mask,
        })
    return in_maps


def run(inputs, trace=False, **spmd_kwargs):
    from concourse import bass_utils

    x, Wq, bq = inputs["x"], inputs["Wq"], inputs["bq"]
    Wk, bk = inputs["Wk"], inputs["bk"]
    Wv, bv = inputs["Wv"], inputs["bv"]
    if (np.any(np.asarray(bq)) or np.any(np.asarray(bk))
            or np.any(np.asarray(bv))):
        return _numpy_fallback(
            np.asarray(x), np.asarray(Wq), np.asarray(bq), np.asarray(Wk),
            np.asarray(bk), np.asarray(Wv), np.asarray(bv)), None

    nc, mask = _get_module()
    in_maps = _make_in_maps(x, Wq, Wk, Wv, mask)
    res = bass_utils.run_bass_kernel_spmd(
        nc, in_maps, core_ids=list(range(B)), trace=trace, **spmd_kwargs,
    )
    out = np.stack(
        [res.results[b]["out"].reshape(HC, WC, C) for b in range(B)]
    ).astype(np.float32)
    return out, res


def kernel(**inputs) -> np.ndarray:
    return run(inputs)[0]


# revision 18
# speedup vs baseline: 1.0098x; 1.0098x over previous
"""v7: scores = x^T (Wq Wk^T) x via on-device M/G precompute (no q/k
projections), 58-col q compaction, 2-pair merged tail ops, consolidated
DMA dispatches, fp16 output. Data-parallel over batch (1 core / element).

Zero-bias fast path: reference.setup_inputs() fixes bq=bk=bv=0. If any
bias is nonzero at runtime we fall back to an exact numpy implementation.
"""

import numpy as np

B, C, H, W, K = 8, 64, 64, 64, 7
HC = WC = H - K + 1          # 58
N = HC * WC                  # 3364
NPAIR = HC // 2              # 29 window-row pairs
NG = 15                      # 14 groups of 2 pairs + 1 single (pair 28)
LAG = 2                      # tail lag in groups
SCALE = float(1.0 / np.sqrt(C))
# slot index in the [128, 8, 116] scores tile for (pair parity, chunk):
# lo-half matmuls (chunks 0, 2) write bank A (slots 0-3), hi-half
# (chunks 1, 3) bank B (slots 4-7).
SLOT = [[0, 4, 1, 5], [2, 6, 3, 7]]

_CACHE = {}


def _pairs_of(g):
    return [2 * g, 2 * g + 1] if g < NG - 1 else [2 * g]


def _build_mask_np():
    """[128, 8, 116] band mask: kk = in-chunk k pixel (rp*64+jp),
    slot = SLOT[pp][c], q = jb*58 + j."""
    m = np.zeros((128, 8, 116), np.float16)
    kk = np.arange(128)
    rp, jp = kk // 64, kk % 64
    jq = np.arange(116)
    jb, j = jq // 58, jq % 58
    for pp in range(2):
        for c in range(4):
            di = 2 * c + rp[:, None] - jb[None, :]
            dj = jp[:, None] - j[None, :]
            ok = (di >= 0) & (di < K) & (dj >= 0) & (dj < K)
            m[:, SLOT[pp][c], :] = ok
    return np.ascontiguousarray(m)


def _build_module():
    import concourse.tile as tile
    from concourse import bacc, mybir

    dt = mybir.dt
    f32 = dt.float32
    f16 = dt.float16

    nc = bacc.Bacc(
        "TRN2", target_bir_lowering=False, debug=False, enable_asserts=False,
        num_devices=8,
    )

    x_d = nc.dram_tensor("x", [C, H, W], f16, kind="ExternalInput").ap()
    wqt_d = nc.dram_tensor("wqT", [C, C], f16, kind="ExternalInput").ap()
    wkt_d = nc.dram_tensor("wkT", [C, C], f16, kind="ExternalInput").ap()
    wv_d = nc.dram_tensor("wv", [C, C], f16, kind="ExternalInput").ap()
    mask_d = nc.dram_tensor("mask", [128, 8, 116], f16, kind="ExternalInput").ap()
    out_d = nc.dram_tensor("out", [N, C], f16, kind="ExternalOutput").ap()

    with tile.TileContext(nc) as tc:
        with (
            tc.tile_pool(name="const", bufs=1) as const,
            tc.tile_pool(name="attn", bufs=2) as attn,
            tc.tile_pool(name="fin", bufs=2) as fin,
            tc.tile_pool(name="ps", bufs=3, space="PSUM") as ps,
        ):
            x_sb = const.tile([128, H, W], f16)      # lo = x, hi = dup
            g_sb = const.tile([128, H, W], f16)      # G = M^T x, lo/hi dup
            v_sb = const.tile([128, 32, C + 1], f16)  # col 64 = ones (denom)
            mask_sb = const.tile([128, 8, 116], f16)
            wqt_sb = const.tile([C, C], f16)
            wkt_sb = const.tile([C, C], f16)
            wv_sb = const.tile([C, C], f16)
            m_sb = const.tile([C, C], f16)
            outt = const.tile([128, NPAIR, C], f16)

            # ---- input DMA dispatches ----
            # Critical-path inputs first (wv, wqT, wkT, x-lo); the bulk
            # transfers (x-hi dup, mask) are dispatched later so they don't
            # hog the DMA engines while the PE waits to start.
            nc.sync.dma_start(wv_sb[:], wv_d[:])
            nc.sync.dma_start(wqt_sb[:], wqt_d[:])
            nc.scalar.dma_start(wkt_sb[:], wkt_d[:])
            nc.sync.dma_start(x_sb[0:64, 0:32, :], x_d[:, 0:32, :])
            nc.sync.dma_start(x_sb[0:64, 32:64, :], x_d[:, 32:64, :])
            nc.gpsimd.memset(v_sb[:], 1.0)
            nc.gpsimd.dma_start(mask_sb[:], mask_d[:])

            # ---- M = Wq Wk^T ----
            pm = ps.tile([128, 8, 128], f32, tag="sc")
            pmf = pm[:].rearrange("p a b -> p (a b)")
            nc.tensor.matmul(pmf[0:64, 0:64], wqt_sb[:], wkt_sb[:])
            nc.scalar.copy(m_sb[:], pmf[0:64, 0:64])
            nc.scalar.dma_start(x_sb[64:128, 0:32, :], x_d[:, 0:32, :])
            nc.scalar.dma_start(x_sb[64:128, 32:64, :], x_d[:, 32:64, :])

            def emit_gchunk(gg):
                # G chunk gg = M^T x rows 8gg..8gg+7, then dup to hi partitions
                pg = ps.tile([128, 8, 128], f32, tag="sc")
                pgf = pg[:].rearrange("p a b -> p (a b)")
                nc.tensor.matmul(
                    pgf[0:64, 0:512], m_sb[:],
                    x_sb[0:64, 8 * gg:8 * gg + 8, :],
                )
                eng = nc.scalar.copy if gg % 2 == 0 else nc.vector.tensor_copy
                eng(g_sb[0:64, 8 * gg:8 * gg + 8, :], pgf[0:64, 0:512])
                nc.sync.dma_start(
                    g_sb[64:128, 8 * gg:8 * gg + 8, :],
                    g_sb[0:64, 8 * gg:8 * gg + 8, :],
                )

            for gg in range(8):
                emit_gchunk(gg)

            sctiles = [None] * NG

            def emit_vpair(r2):
                # project v for 2-row-pair chunks 2*r2, 2*r2+1
                pv = ps.tile([128, 2, C + 1], f32, tag="out", bufs=2)
                for h in range(2):
                    r = 2 * r2 + h
                    nc.tensor.matmul(
                        pv[:, h, 0:C],
                        x_sb[0:64, 2 * r:2 * r + 2, :],
                        wv_sb[:],
                    )
                eng = nc.scalar.copy if r2 % 2 == 0 else nc.vector.tensor_copy
                eng(v_sb[:, 2 * r2:2 * r2 + 2, 0:C], pv[:, :, 0:C])

            def emit_scores(g):
                sc = ps.tile([128, 8, 128], f32, tag="sc")
                sctiles[g] = sc
                for pp, p in enumerate(_pairs_of(g)):
                    i = 2 * p
                    glo = g_sb[0:64, i + 3:i + 5, 3:61]
                    ghi = g_sb[64:128, i + 3:i + 5, 3:61]
                    for c in range(4):
                        if c % 2 == 0:
                            nc.tensor.matmul(
                                sc[:, SLOT[pp][c], 0:116],
                                x_sb[0:64, i + 2 * c:i + 2 * c + 2, :],
                                glo,
                            )
                        else:
                            nc.tensor.matmul(
                                sc[:, SLOT[pp][c], 0:116],
                                x_sb[64:128, i + 2 * c:i + 2 * c + 2, :],
                                ghi,
                            )

            def emit_tail(g):
                sc = sctiles[g]
                pairs = _pairs_of(g)
                ex = attn.tile([128, 8, 116], f16, tag="ex")
                if len(pairs) == 2:
                    nc.scalar.activation(
                        ex[:], sc[:, :, 0:116],
                        mybir.ActivationFunctionType.Exp, scale=SCALE,
                    )
                else:
                    sc_h = sc[:].rearrange("p (h s) q -> p h s q", h=2)[:, :, 0:2, 0:116]
                    ex_h = ex[:].rearrange("p (h s) q -> p h s q", h=2)[:, :, 0:2, :]
                    nc.scalar.activation(
                        ex_h, sc_h,
                        mybir.ActivationFunctionType.Exp, scale=SCALE,
                    )
                at = attn.tile([128, 8, 116], f16, tag="at")
                if len(pairs) == 2:
                    nc.vector.tensor_mul(at[:], ex[:], mask_sb[:])
                else:
                    def _h(ap):
                        return ap.rearrange(
                            "p (h s) q -> p h s q", h=2)[:, :, 0:2, :]
                    nc.vector.tensor_mul(_h(at[:]), _h(ex[:]), _h(mask_sb[:]))
                ops = ps.tile([128, 2, C + 1], f32, tag="out", bufs=2)
                for pp, p in enumerate(pairs):
                    for c in range(4):
                        nc.tensor.matmul(
                            ops[0:116, pp, :],
                            at[:, SLOT[pp][c], :],
                            v_sb[:, p + c, :],
                            start=(c == 0), stop=(c == 3),
                        )
                rc = fin.tile([128, 2], f32, tag="rc")
                nc.vector.reciprocal(
                    rc[0:116, 0:len(pairs)],
                    ops[0:116, 0:len(pairs), C:C + 1].rearrange("p a b -> p (a b)"),
                )
                for pp, p in enumerate(pairs):
                    nc.vector.tensor_scalar(
                        outt[0:116, p:p + 1, :],
                        ops[0:116, pp:pp + 1, 0:C],
                        rc[0:116, pp:pp + 1], None,
                        mybir.AluOpType.mult,
                    )

            def emit_outdma(p0, p1):
                npair = p1 - p0
                dst = out_d[p0 * 2 * WC:p1 * 2 * WC, :].rearrange(
                    "(p jb j) c -> (jb j) p c", jb=2, j=WC,
                )
                nc.sync.dma_start(dst, outt[0:116, p0:p1, :])

            # all v projections run in the front / first two iterations so
            # the "out" psum ring alternates cleanly between ops tiles once
            # tails start (vp allocations interleaved with ops would pin
            # ops to one slot, serializing av(g+1) on muls(g)).
            for r2 in range(4):
                emit_vpair(r2)

            for g in range(NG):
                if g == 0:
                    for r2 in range(4, 10):
                        emit_vpair(r2)
                if g == 1:
                    for r2 in range(10, 16):
                        emit_vpair(r2)
                emit_scores(g)
                if g >= LAG:
                    emit_tail(g - LAG)
                if g == 8:
                    emit_outdma(0, 14)
                if g == 13:
                    emit_outdma(14, 24)
            for g in range(NG - LAG, NG):
                emit_tail(g)
            emit_outdma(24, NPAIR)

    nc.compile()
    return nc


def _get_module():
    if "nc" not in _CACHE:
        _CACHE["nc"] = _build_module()
        _CACHE["mask"] = _build_mask_np()
    return _CACHE["nc"], _CACHE["mask"]


def _numpy_fallback(x, Wq, bq, Wk, bk, Wv, bv):
    xf = np.transpose(x, (0, 2, 3, 1)).reshape(B, H * W, C).astype(np.float64)
    ii, jj = np.meshgrid(np.arange(HC), np.arange(WC), indexing="ij")
    di, dj = np.meshgrid(np.arange(K), np.arange(K), indexing="ij")
    rows = ii.reshape(-1, 1) + di.reshape(1, -1)
    cols = jj.reshape(-1, 1) + dj.reshape(1, -1)
    idx = (rows * W + cols).reshape(-1)
    KK = K * K
    patches = xf[:, idx, :].reshape(B, N, KK, C)
    q = patches[:, :, KK // 2, :] @ Wq + bq
    k = patches @ Wk + bk
    v = patches @ Wv + bv
    s = np.einsum("bnc,bnkc->bnk", q, k) * SCALE
    a = np.exp(s - s.max(-1, keepdims=True))
    a /= a.sum(-1, keepdims=True)
    out = np.einsum("bnk,bnkc->bnc", a, v)
    return out.reshape(B, HC, WC, C).astype(np.float32)


def _make_in_maps(x, Wq, Wk, Wv, mask):
    wqt = np.ascontiguousarray(np.asarray(Wq).T).astype(np.float16)
    wkt = np.ascontiguousarray(np.asarray(Wk).T).astype(np.float16)
    wv = np.ascontiguousarray(np.asarray(Wv)).astype(np.float16)
    in_maps = []
    for b in range(B):
        in_maps.append({
            "x": np.ascontiguousarray(np.asarray(x[b]).astype(np.float16)),
            "wqT": wqt, "wkT": wkt, "wv": wv,
            "mask": mask,
        })
    return in_maps


def run(inputs, trace=False, **spmd_kwargs):
    from concourse import bass_utils

    x, Wq, bq = inputs["x"], inputs["Wq"], inputs["bq"]
    Wk, bk = inputs["Wk"], inputs["bk"]
    Wv, bv = inputs["Wv"], inputs["bv"]
    if (np.any(np.asarray(bq)) or np.any(np.asarray(bk))
            or np.any(np.asarray(bv))):
        return _numpy_fallback(
            np.asarray(x), np.asarray(Wq), np.asarray(bq), np.asarray(Wk),
            np.asarray(bk), np.asarray(Wv), np.asarray(bv)), None

    nc, mask = _get_module()
    in_maps = _make_in_maps(x, Wq, Wk, Wv, mask)
    res = bass_utils.run_bass_kernel_spmd(
        nc, in_maps, core_ids=list(range(B)), trace=trace, **spmd_kwargs,
    )
    out = np.stack(
        [res.results[b]["out"].reshape(HC, WC, C) for b in range(B)]
    ).astype(np.float32)
    return out, res


def kernel(**inputs) -> np.ndarray:
    return run(inputs)[0]
